# revision 1
# baseline (speedup 1.0000x reference)
"""Trainium2 Bass kernel for nn_ExpandedTerrainFeatures.

Input: foot/shank/thigh [16384, 12, 256] f32. Output: [16384, 208] f32.
Pure data-parallel across 8 NeuronCores (2048 samples each); inside a core,
16 tiles of 128 samples (partition dim = sample).

Feature blocks per tile (see build_tile):
  0..95    summary stats of 12 channel-group norms (8 each, signal-major)
  96..123  spectral feats of 4 z-signals (PE-matmul DFT power spectrum)
  124..171 heel/toe phase features (cumsum window sums around abs-argmax)
  172..183 foot-shank coupling (direct 17-lag xcorr)
  184..195 horizontal-norm features
  196..207 asymmetry log-ratios
"""
import sys, os
import numpy as np

for _p in ("/opt/trn_rl_repo",):
    if _p not in sys.path and os.path.isdir(_p):
        sys.path.insert(0, _p)

import concourse.bass as bass
import concourse.tile as tile
from concourse import bacc, mybir
from concourse.bass_utils import run_bass_kernel_spmd

F32 = mybir.dt.float32
U32 = mybir.dt.uint32
AF = mybir.ActivationFunctionType
OP = mybir.AluOpType
AX = mybir.AxisListType

T = 256
EPS = 1e-6
NSIG = 12

# IQR probe constants (validated offline on randn data):
# probe1 at mean + c1*sd (s domain), probe2 shifts by alpha*sd per count-miss,
# aiming count(<=v2) at k+CENT so the needed ranks sit inside the top-32
# of the masked set.
IQR_CFG = {64: (-0.75, 0.009, 16.0), 192: (0.65, 0.0105, 18.0)}
CHAIN = 32  # top-k chain depth (4x max8 + 3x match_replace)

# spectral constants
NBIN = 130  # 129 rfft bins + 1 zero pad
BAND_SLICES = [(0, 8), (8, 16), (16, 26), (26, 52), (52, 103)]
FSTEP = 100.0 / 256.0

# phase segments: (offset, length, R)
HEEL = (0, 115, 19)
TOE = (153, 103, 17)

LAGS = 8  # xcorr max lag
STOP_AFTER = None  # debug: truncate build_tile after N sections


def _consts():
    k = np.arange(NBIN)
    t = np.arange(T)
    wc = np.cos(-2 * np.pi * np.outer(t, k) / T).astype(np.float32)
    ws = np.sin(-2 * np.pi * np.outer(t, k) / T).astype(np.float32)
    wc[:, 129] = 0.0
    ws[:, 129] = 0.0
    W = np.concatenate([wc, ws], 1)  # [256, 260]
    Wr = np.ascontiguousarray(W.reshape(2, 128, 2 * NBIN).transpose(1, 0, 2))  # [t_in_chunk, chunk, col]
    ident = np.eye(128, dtype=np.float32)
    iota_iqr = np.tile(np.arange(32, dtype=np.float32), (128, 12, 1))
    iota_ph = np.tile(np.arange(115, dtype=np.float32), (128, 4, 1))
    return Wr, ident, iota_iqr, iota_ph


def build_tile(tc, pools, consts, ins, out_d, ti):
    """Emit instructions for one [128, ...] sample tile."""
    nc = tc.nc
    iosb, psum, work, small = pools
    W_sb, id_sb, eps_sb, zeros_sb, iota_iqr_sb, iota_ph_sb = consts
    foot_d, shank_d, thigh_d = ins
    P = 128
    r0 = ti * P

    def tsplit(ap):  # [128, 12, 256] -> grouped view helper
        return ap

    # ---- load inputs ------------------------------------------------------
    xs = []
    for name, src in (("foot", foot_d), ("shank", shank_d), ("thigh", thigh_d)):
        t_ = iosb.tile([P, 12, T], F32, tag=name)
        nc.sync.dma_start(t_[:], src[r0:r0 + P])
        xs.append(t_)
    foot_sb, shank_sb, thigh_sb = xs

    out_sb = iosb.tile([P, 208], F32, tag="out")
    if STOP_AFTER is not None:
        nc.vector.memset(out_sb[:], 0.0)

    _sec = [0]

    def _cut():
        _sec[0] += 1
        if STOP_AFTER is not None and _sec[0] >= STOP_AFTER:
            nc.sync.dma_start(out_d[r0:r0 + P], out_sb[:])
            return True
        return False

    # ---- squares + group norms -------------------------------------------
    # Signal order per tensor: (a_lt, g_lt, a_rt, g_rt) [_norms4 natural
    # order]; output writes go through a permuted view to match the
    # reference's (a_lt, a_rt, g_lt, g_rt).
    nsqa = work.tile([P, NSIG, T], F32, tag="nsqa", bufs=2)
    sq_foot = work.tile([P, 12, T], F32, tag="sqf")
    nc.scalar.square(sq_foot[:], foot_sb[:])
    vf = sq_foot[:].rearrange("p (g c) t -> p g c t", c=3)  # [p,4grp,3,T]
    tf = work.tile([P, 4, T], F32, tag="tf", bufs=1)
    nc.vector.tensor_tensor(tf[:], vf[:, :, 0, :], vf[:, :, 1, :], OP.add)
    nc.vector.tensor_tensor(nsqa[:, 0:4, :], tf[:], vf[:, :, 2, :], OP.add)
    for xi, x_sb in ((1, shank_sb), (2, thigh_sb)):
        for hf in range(2):  # side halves: 6 channels = 2 groups
            sq6 = work.tile([P, 6, T], F32, tag="sqo", bufs=2)
            nc.scalar.square(sq6[:], x_sb[:, 6 * hf:6 * hf + 6, :])
            v6 = sq6[:].rearrange("p (g c) t -> p g c t", c=3)  # [p,2,3,T]
            o = 4 * xi + 2 * hf
            t6 = work.tile([P, 2, T], F32, tag="t6", bufs=1)
            nc.vector.tensor_tensor(t6[:], v6[:, :, 0, :], v6[:, :, 1, :], OP.add)
            nc.vector.tensor_tensor(nsqa[:, o:o + 2, :], t6[:], v6[:, :, 2, :], OP.add)
    s12 = work.tile([P, NSIG, T], F32, tag="s12", bufs=1)
    nc.scalar.activation(s12[:], nsqa[:], AF.Sqrt)

    if _cut():
        return
    # ---- summary: mean/var + high moments --------------------------------
    def mean_var(src, nseg, seglen, tag):
        """bn_stats/bn_aggr: src [P, nseg, seglen] -> [P, nseg, 2] (mean, var)"""
        st6 = small.tile([P, nseg, 6], F32, tag=tag + "6")
        for s0 in range(nseg):
            nc.vector.bn_stats(st6[:, s0, :], src[:, s0, :])
        st2 = small.tile([P, nseg, 2], F32, tag=tag + "2")
        for s in range(nseg):
            nc.vector.bn_aggr(st2[:, s, :], st6[:, s, :])
        return st2

    bn2 = mean_var(s12, NSIG, T, "bn")
    mean = bn2[:, :, 0]  # [P,12] strided views
    var = bn2[:, :, 1]

    acc3 = small.tile([P, NSIG], F32, tag="acc3")
    acc4 = small.tile([P, NSIG], F32, tag="acc4")
    for s in range(NSIG):
        junk = work.tile([P, T], F32, tag="junk", bufs=4)
        # sum(s^3) = sum(nsq * s)
        nc.vector.scalar_tensor_tensor(junk[:], nsqa[:, s, :], 1.0, s12[:, s, :],
                                       OP.mult, OP.mult, accum_out=acc3[:, s:s + 1])
        # sum(nsq^2) = sum(s^4)
        nc.scalar.activation(work.tile([P, T], F32, tag="junk", name="junka", bufs=4)[:],
                             nsqa[:, s, :], AF.Square,
                             accum_out=acc4[:, s:s + 1])

    def sm(tag, shape=(P, NSIG)):
        return small.tile(list(shape), F32, tag=tag, name=tag)

    mm = sm("mm"); nc.vector.tensor_tensor(mm[:], mean, mean, OP.mult)
    e2 = sm("e2"); nc.vector.tensor_tensor(e2[:], var, mm[:], OP.add)
    e3 = sm("e3"); nc.vector.tensor_scalar(e3[:], acc3[:], 1.0 / T, None, OP.mult)
    e4 = sm("e4"); nc.vector.tensor_scalar(e4[:], acc4[:], 1.0 / T, None, OP.mult)
    # m3 = e3 - m*(3e2 - 2mm)
    t1 = sm("t1"); nc.vector.tensor_scalar(t1[:], mm[:], -2.0, None, OP.mult)
    t1b = sm("t1b"); nc.vector.scalar_tensor_tensor(t1b[:], e2[:], 3.0, t1[:], OP.mult, OP.add)
    t2 = sm("t2"); nc.vector.tensor_tensor(t2[:], t1b[:], mean, OP.mult)
    m3 = sm("m3"); nc.vector.tensor_tensor(m3[:], e3[:], t2[:], OP.subtract)
    # m4 = e4 - 4m*e3 + 6mm*e2 - 3mm^2
    u1 = sm("u1"); nc.vector.scalar_tensor_tensor(u1[:], e3[:], -4.0, mean, OP.mult, OP.mult)
    u2 = sm("u2"); nc.vector.scalar_tensor_tensor(u2[:], e2[:], 6.0, mm[:], OP.mult, OP.mult)
    u3 = sm("u3"); nc.vector.scalar_tensor_tensor(u3[:], mm[:], -3.0, mm[:], OP.mult, OP.mult)
    m4 = sm("m4"); nc.vector.tensor_tensor(m4[:], e4[:], u1[:], OP.add)
    nc.vector.tensor_tensor(m4[:], m4[:], u2[:], OP.add)
    nc.vector.tensor_tensor(m4[:], m4[:], u3[:], OP.add)

    varc = sm("varc"); nc.vector.tensor_scalar(varc[:], var, EPS, None, OP.max)
    rvar = sm("rvar"); nc.vector.reciprocal(rvar[:], varc[:])
    sdq = sm("sdq"); nc.scalar.activation(sdq[:], varc[:], AF.Sqrt)

    # write view: permutes (quant, side) -> my (side, quant) signal order
    osum5 = out_sb[:, 0:96].rearrange("p (k a b f) -> p k b a f", k=3, a=2, b=2, f=8)
    OF = lambda f: osum5[:, :, :, :, f]
    P4 = lambda ap: ap.rearrange("p (k s q) -> p k s q", k=3, s=2)
    # read view in reference signal order
    osumR = out_sb[:, 0:96].rearrange("p (s f) -> p s f", f=8)
    nc.scalar.copy(OF(0), P4(mean))                                  # mean
    nc.scalar.activation(OF(1), P4(var), AF.Sqrt, scale=T / (T - 1.0))  # std
    nc.scalar.activation(OF(2), P4(e2[:]), AF.Sqrt)                  # rms
    # skew = clip(m3 * sdq * rvar^2, +-10)
    sk = sm("sk"); nc.vector.tensor_tensor(sk[:], m3[:], sdq[:], OP.mult)
    nc.vector.tensor_tensor(sk[:], sk[:], rvar[:], OP.mult)
    nc.vector.tensor_tensor(sk[:], sk[:], rvar[:], OP.mult)
    nc.vector.tensor_scalar(sk[:], sk[:], -10.0, 10.0, OP.max, OP.min)
    nc.scalar.copy(OF(6), P4(sk[:]))
    ku = sm("ku"); nc.vector.tensor_tensor(ku[:], m4[:], rvar[:], OP.mult)
    nc.vector.tensor_tensor(ku[:], ku[:], rvar[:], OP.mult)
    nc.vector.tensor_scalar(ku[:], ku[:], 0.0, 30.0, OP.max, OP.min)
    nc.scalar.copy(OF(7), P4(ku[:]))

    if _cut():
        return
    # ---- q95 via top-16 of nsqa ------------------------------------------
    top16 = small.tile([P, NSIG, 16], F32, tag="top16")
    for s in range(NSIG):
        rep = work.tile([P, T], F32, tag="u0", bufs=2)
        nc.vector.max(top16[:, s, 0:8], nsqa[:, s, :])
        nc.vector.match_replace(rep[:], top16[:, s, 0:8], nsqa[:, s, :], -1.0)
        nc.vector.max(top16[:, s, 8:16], rep[:])

    if _cut():
        return
    # ---- IQR via 2 probes + depth-32 chain -------------------------------
    qsel = small.tile([P, NSIG, 4], F32, tag="qsel")  # s63,s64,s191,s192 (nsqa units)
    for qi, kk in enumerate((64, 192)):
        c1, alpha, cent = IQR_CFG[kk]
        v1s = sm("v1s_%d" % kk)
        nc.vector.scalar_tensor_tensor(v1s[:], sdq[:], c1, mean, OP.mult, OP.add)
        nc.scalar.activation(v1s[:], v1s[:], AF.Relu)
        v1 = sm("v1_%d" % kk)
        nc.scalar.activation(v1[:], v1s[:], AF.Square)
        cnt1 = sm("cnt1_%d" % kk)
        for s in range(NSIG):
            nc.vector.tensor_scalar(work.tile([P, T], F32, tag="junk", name="junkb", bufs=4)[:],
                                    nsqa[:, s, :], v1[:, s:s + 1], None, OP.is_le,
                                    op1=OP.add, accum_out=cnt1[:, s:s + 1])
        d = sm("d_%d" % kk)
        nc.vector.tensor_scalar(d[:], cnt1[:], float(kk) + cent, -alpha,
                                OP.subtract, OP.mult)
        v2s = sm("v2s_%d" % kk)
        nc.vector.tensor_tensor(v2s[:], d[:], sdq[:], OP.mult)
        nc.vector.tensor_tensor(v2s[:], v2s[:], v1s[:], OP.add)
        nc.scalar.activation(v2s[:], v2s[:], AF.Relu)
        v2 = sm("v2_%d" % kk)
        nc.scalar.activation(v2[:], v2s[:], AF.Square)
        cnt2 = sm("cnt2_%d" % kk)
        chain = small.tile([P, NSIG, CHAIN], F32, tag="chain_%d" % kk)
        for s in range(NSIG):
            u = work.tile([P, T], F32, tag="u0", bufs=2)
            nc.vector.tensor_scalar(u[:], nsqa[:, s, :], v2[:, s:s + 1], None,
                                    OP.is_le, op1=OP.add,
                                    accum_out=cnt2[:, s:s + 1])
            nc.vector.tensor_tensor(u[:], u[:], nsqa[:, s, :], OP.mult)
            cur = u
            for stage in range(4):
                nc.vector.max(chain[:, s, 8 * stage:8 * stage + 8], cur[:])
                if stage < 3:
                    nxt = work.tile([P, T], F32, tag="u%d" % (1 + stage % 2), bufs=2)
                    nc.vector.match_replace(nxt[:], chain[:, s, 8 * stage:8 * stage + 8],
                                            cur[:], -1.0)
                    cur = nxt
        # j_lo selects rank kk-1, j_hi rank kk (desc idx = cnt2-kk / cnt2-kk-1)
        jlo = sm("jlo_%d" % kk)
        nc.vector.tensor_scalar(jlo[:], cnt2[:], float(kk), 0.0, OP.subtract, OP.max)
        nc.vector.tensor_scalar(jlo[:], jlo[:], float(CHAIN - 1), None, OP.min)
        jhi = sm("jhi_%d" % kk)
        nc.vector.tensor_scalar(jhi[:], cnt2[:], float(kk) + 1.0, 0.0, OP.subtract, OP.max)
        nc.vector.tensor_scalar(jhi[:], jhi[:], float(CHAIN - 1), None, OP.min)
        for jj, jt in ((0, jlo), (1, jhi)):
            oh = work.tile([P, NSIG, CHAIN], F32, tag="oh", bufs=1)
            nc.vector.tensor_tensor(oh[:], iota_iqr_sb[:],
                                    jt[:].unsqueeze(2).broadcast_to((P, NSIG, CHAIN)),
                                    OP.is_equal)
            nc.vector.tensor_tensor(oh[:], oh[:], chain[:], OP.mult)
            nc.vector.tensor_reduce(qsel[:, :, 2 * qi + jj], oh[:], AX.X, OP.add)

    roots = small.tile([P, NSIG, 7], F32, tag="roots")
    nc.scalar.copy(roots[:, :, 0:4], qsel[:])
    nc.scalar.copy(roots[:, :, 4], top16[:, :, 13])
    nc.scalar.copy(roots[:, :, 5], top16[:, :, 12])
    nc.scalar.copy(roots[:, :, 6], top16[:, :, 0])
    nc.scalar.activation(roots[:], roots[:], AF.Sqrt)
    # lerps: q25 = r0+0.75(r1-r0); q75 = r2+0.25(r3-r2); q95 = r4+0.25(r5-r4)
    q25 = sm("q25"); q75 = sm("q75")
    dq = sm("dq")
    nc.vector.tensor_tensor(dq[:], roots[:, :, 1], roots[:, :, 0], OP.subtract)
    nc.vector.scalar_tensor_tensor(q25[:], dq[:], 0.75, roots[:, :, 0], OP.mult, OP.add)
    nc.vector.tensor_tensor(dq[:], roots[:, :, 3], roots[:, :, 2], OP.subtract)
    nc.vector.scalar_tensor_tensor(q75[:], dq[:], 0.25, roots[:, :, 2], OP.mult, OP.add)
    iqr_t = sm("iqr_t"); nc.vector.tensor_tensor(iqr_t[:], q75[:], q25[:], OP.subtract)
    nc.scalar.copy(OF(5), P4(iqr_t[:]))  # IQR
    nc.vector.tensor_tensor(dq[:], roots[:, :, 5], roots[:, :, 4], OP.subtract)
    q95_t = sm("q95_t")
    nc.vector.scalar_tensor_tensor(q95_t[:], dq[:], 0.25, roots[:, :, 4],
                                   OP.mult, OP.add)
    nc.scalar.copy(OF(4), P4(q95_t[:]))                                  # q95
    nc.scalar.copy(OF(3), P4(roots[:, :, 6]))                        # max

    if _cut():
        return
    # ---- z4 slices --------------------------------------------------------
    zf = foot_sb[:].rearrange("p (g s) t -> p g s t", s=6)[:, :, 2, :]   # [P,2,T]
    zs = shank_sb[:].rearrange("p (g s) t -> p g s t", s=6)[:, :, 2, :]
    zviews = [zf[:, 0, :], zf[:, 1, :], zs[:, 0, :], zs[:, 1, :]]

    # ---- spectral ---------------------------------------------------------
    SPv = out_sb[:, 96:124].rearrange("p (s f) -> p s f", f=7)  # [P,4,7]
    pwr = work.tile([P, 4, NBIN], F32, tag="pwr")
    for s in range(4):
        xT = work.tile([P, 2, 128], F32, tag="xT")
        for c in range(2):
            tp = psum.tile([P, 128], F32, tag="tp")
            nc.tensor.transpose(tp[:], zviews[s][:, 128 * c:128 * (c + 1)], id_sb[:])
            nc.scalar.copy(xT[:, c, :], tp[:])
        dft = psum.tile([P, 2 * NBIN], F32, tag="dft")
        for c in range(2):
            nc.tensor.matmul(dft[:], xT[:, c, :], W_sb[:, c, :],
                             start=(c == 0), stop=(c == 1))
        im2 = work.tile([P, NBIN], F32, tag="im2")
        nc.scalar.activation(pwr[:, s, :], dft[:, 0:NBIN], AF.Square)
        nc.scalar.activation(im2[:], dft[:, NBIN:2 * NBIN], AF.Square)
        nc.vector.tensor_tensor(pwr[:, s, :], pwr[:, s, :], im2[:], OP.add)
    tot = small.tile([P, 4], F32, tag="tot")
    nc.vector.tensor_reduce(tot[:], pwr[:, :, 0:129], AX.X, OP.add)
    nc.vector.tensor_scalar(tot[:], tot[:], 1e-8, None, OP.max)
    rtot = small.tile([P, 4], F32, tag="rtot")
    nc.vector.reciprocal(rtot[:], tot[:])
    for j, (lo, hi) in enumerate(BAND_SLICES):
        nc.vector.tensor_reduce(SPv[:, :, j], pwr[:, :, lo:hi], AX.X, OP.add)
    nc.vector.tensor_tensor(SPv[:, :, 0:5], SPv[:, :, 0:5],
                            rtot[:].unsqueeze(2).broadcast_to((P, 4, 5)), OP.mult)
    # rolloff (before pn overwrites pwr in place)
    thr = small.tile([P, 4], F32, tag="thr")
    nc.vector.tensor_scalar(thr[:], tot[:], 0.85, None, OP.mult)
    for s in range(4):
        cum = work.tile([P, NBIN], F32, tag="cum", bufs=1)
        nc.vector.tensor_tensor_scan(cum[:], pwr[:, s, :], zeros_sb[:, 0:NBIN], 0.0,
                                     OP.add, OP.add)
        nc.vector.tensor_scalar(work.tile([P, NBIN], F32, tag="junk2", name="junkd", bufs=4)[:],
                                cum[:], thr[:, s:s + 1], None, OP.is_lt,
                                op1=OP.add, accum_out=SPv[:, s, 6:7])
    nc.vector.tensor_scalar(SPv[:, :, 6], SPv[:, :, 6], FSTEP, None, OP.mult)
    # entropy (pn overwrites pwr)
    pn = pwr
    for s in range(4):
        nc.scalar.activation(pn[:, s, :], pwr[:, s, :], AF.Copy, scale=rtot[:, s:s + 1])
    nc.vector.tensor_scalar(pn[:], pn[:], 1e-8, None, OP.max)
    lnp = work.tile([P, 4, NBIN], F32, tag="lnp")
    nc.scalar.activation(lnp[:], pn[:], AF.Ln)
    ent = small.tile([P, 4], F32, tag="ent")
    for s in range(4):
        nc.vector.scalar_tensor_tensor(work.tile([P, NBIN], F32, tag="junk2", name="junkc", bufs=4)[:],
                                       pn[:, s, :], 1.0, lnp[:, s, :],
                                       OP.mult, OP.mult, accum_out=ent[:, s:s + 1])
    # remove padded-bin contribution 1e-8*ln(1e-8), scale by -1/ln(130)
    _padfix = 1e-8 * float(np.log(1e-8))
    nc.vector.tensor_scalar(SPv[:, :, 5], ent[:], -_padfix, -1.0 / float(np.log(130.0)),
                            OP.subtract, OP.mult)

    if _cut():
        return
    # ---- phase features (heel, toe) --------------------------------------
    for pi, (off, sT, R) in enumerate((HEEL, TOE)):
        base = 124 + 24 * pi
        Hv = out_sb[:, base:base + 24].rearrange("p (s f) -> p s f", f=6)
        PL = 1 + sT + 2 * R
        seg_f = zf[:, :, off:off + sT]
        seg_s = zs[:, :, off:off + sT]
        pad = work.tile([P, 4, PL], F32, tag="pad")
        nc.vector.memset(pad[:, :, 0:1], 0.0)
        nc.scalar.activation(pad[:, 0:2, 1 + R:1 + R + sT], seg_f, AF.Abs)
        nc.scalar.activation(pad[:, 2:4, 1 + R:1 + R + sT], seg_s, AF.Abs)
        nc.scalar.copy(pad[:, :, 1:1 + R],
                       pad[:, :, 1 + R:2 + R].broadcast_to((P, 4, R)))
        nc.scalar.copy(pad[:, :, 1 + R + sT:PL],
                       pad[:, :, R + sT:R + sT + 1].broadcast_to((P, 4, R)))
        # max + argmax over sa = pad middle
        mx8 = small.tile([P, 4, 8], F32, tag="mx8")
        ix8 = small.tile([P, 4, 8], U32, tag="ix8")
        for s in range(4):
            nc.vector.max(mx8[:, s, :], pad[:, s, 1 + R:1 + R + sT])
            nc.vector.max_index(ix8[:, s, :], mx8[:, s, :], pad[:, s, 1 + R:1 + R + sT])
        mx = small.tile([P, 4], F32, tag="mx")
        nc.scalar.copy(mx[:], mx8[:, :, 0])
        idxf = small.tile([P, 4], F32, tag="idxf")
        nc.vector.tensor_copy(idxf[:], ix8[:, :, 0])
        # cumsums (pad includes leading zero)
        cz = work.tile([P, 4, PL], F32, tag="cz")
        for s in range(4):
            nc.vector.tensor_tensor_scan(cz[:, s, :], pad[:, s, :], zeros_sb[:, 0:PL],
                                         0.0, OP.add, OP.add)
        # count mask >= 0.2*mx
        thr2 = small.tile([P, 4], F32, tag="thr2")
        nc.vector.tensor_scalar(thr2[:], mx[:], 0.2, None, OP.mult)
        cm = pad  # overwrite in place: pad has no readers after this
        nc.vector.tensor_tensor(cm[:], pad[:],
                                thr2[:].unsqueeze(2).broadcast_to((P, 4, PL)), OP.is_ge)
        nc.vector.memset(cm[:, :, 0:1], 0.0)
        cc = work.tile([P, 4, PL], F32, tag="cc")
        for s in range(4):
            nc.vector.tensor_tensor_scan(cc[:, s, :], cm[:, s, :], zeros_sb[:, 0:PL],
                                         0.0, OP.add, OP.add)
        # windowed sums (at every t), then select at idx via onehot dot
        preS = work.tile([P, 4, sT], F32, tag="preS")
        nc.vector.tensor_tensor(preS[:], cz[:, :, R:R + sT], cz[:, :, 0:sT], OP.subtract)
        postS = work.tile([P, 4, sT], F32, tag="postS")
        nc.vector.tensor_tensor(postS[:], cz[:, :, 2 * R + 1:2 * R + 1 + sT],
                                cz[:, :, R + 1:R + 1 + sT], OP.subtract)
        cntS = work.tile([P, 4, sT], F32, tag="cntS")
        nc.vector.tensor_tensor(cntS[:], cc[:, :, 2 * R + 1:2 * R + 1 + sT],
                                cc[:, :, 0:sT], OP.subtract)
        oh = work.tile([P, 4, sT], F32, tag="ohp")
        nc.vector.tensor_tensor(oh[:], iota_ph_sb[:, :, 0:sT],
                                idxf[:].unsqueeze(2).broadcast_to((P, 4, sT)), OP.is_equal)
        sel = small.tile([P, 4, 3], F32, tag="selp")
        for j, q in enumerate((preS, postS, cntS)):
            tmp = work.tile([P, 4, sT], F32, tag="ohtmp", bufs=1)
            nc.vector.tensor_tensor(tmp[:], oh[:], q[:], OP.mult)
            nc.vector.tensor_reduce(sel[:, :, j], tmp[:], AX.X, OP.add)
        # features
        nc.scalar.copy(Hv[:, :, 0], mx[:])                       # pk
        locs = small.tile([P, 4], F32, tag="locs")
        nc.vector.tensor_tensor(locs[:], sel[:, :, 0], sel[:, :, 1], OP.add)
        nc.vector.tensor_tensor(Hv[:, :, 1], locs[:], mx[:], OP.add)  # loc sum
        pr = small.tile([P, 4], F32, tag="pr")
        nc.vector.tensor_scalar(pr[:], sel[:, :, 0], 1.0 / R, EPS, OP.mult, OP.add)
        nc.vector.reciprocal(pr[:], pr[:])
        po = small.tile([P, 4], F32, tag="po")
        nc.vector.tensor_scalar(po[:], sel[:, :, 1], 1.0 / R, None, OP.mult)
        nc.vector.tensor_tensor(Hv[:, :, 2], po[:], pr[:], OP.mult)  # post/pre
        nc.vector.tensor_scalar(Hv[:, :, 3], sel[:, :, 2], 1.0 / (2 * R + 1), None,
                                OP.mult)                              # frac
        # jerk
        jk = work.tile([P, 4, sT - 1], F32, tag="jk")
        nc.vector.tensor_tensor(jk[:, 0:2, :], seg_f[:, :, 1:], seg_f[:, :, :-1], OP.subtract)
        nc.vector.tensor_tensor(jk[:, 2:4, :], seg_s[:, :, 1:], seg_s[:, :, :-1], OP.subtract)
        nc.vector.tensor_reduce(Hv[:, :, 4], jk[:], AX.X, OP.max,
                                apply_absolute_value=True)            # |jerk|max
        jb2 = mean_var(jk, 4, sT - 1, "jb")
        jmm = small.tile([P, 4], F32, tag="jmm")
        nc.vector.tensor_tensor(jmm[:], jb2[:, :, 0], jb2[:, :, 0], OP.mult)
        nc.vector.tensor_tensor(jmm[:], jmm[:], jb2[:, :, 1], OP.add)
        nc.scalar.activation(Hv[:, :, 5], jmm[:], AF.Sqrt)            # jerk rms

    if _cut():
        return
    # ---- xcorr + coupling -------------------------------------------------
    zbn6 = small.tile([P, 4, 6], F32, tag="zbn6")
    for s in range(2):
        nc.vector.bn_stats(zbn6[:, s, :], zf[:, s, :])
        nc.vector.bn_stats(zbn6[:, 2 + s, :], zs[:, s, :])
    zbn2 = small.tile([P, 4, 2], F32, tag="zbn2")
    for s in range(4):
        nc.vector.bn_aggr(zbn2[:, s, :], zbn6[:, s, :])
    negm = small.tile([P, 4], F32, tag="negm")
    nc.vector.tensor_scalar(negm[:], zbn2[:, :, 0], -1.0, None, OP.mult)
    x04 = work.tile([P, 4, T], F32, tag="x04")
    for s in range(4):
        nc.scalar.activation(x04[:, s, :], zviews[s], AF.Identity,
                             bias=negm[:, s:s + 1])
    corr = small.tile([P, 2, 17], F32, tag="corr")
    for p_ in range(2):
        fz, sz = p_, p_ + 2
        for j, l in enumerate(range(-LAGS, LAGS + 1)):
            a0, b0 = max(0, l), max(0, -l)
            n = T - abs(l)
            nc.vector.scalar_tensor_tensor(
                work.tile([P, T], F32, tag="junk", name="junke", bufs=4)[:, 0:n],
                x04[:, fz, a0:a0 + n], 1.0, x04[:, sz, b0:b0 + n],
                OP.mult, OP.mult, accum_out=corr[:, p_, j:j + 1])
    cmax = small.tile([P, 2], F32, tag="cmax")
    nc.vector.tensor_reduce(cmax[:], corr[:], AX.X, OP.max)
    ohc = small.tile([P, 2, 17], F32, tag="ohc")
    nc.vector.tensor_tensor(ohc[:], corr[:],
                            cmax[:].unsqueeze(2).broadcast_to((P, 2, 17)), OP.is_equal)
    wc_ = small.tile([P, 2, 17], F32, tag="wc")
    nc.vector.tensor_tensor(wc_[:], ohc[:],
                            iota_ph_sb[:, 0:2, 0:17], OP.mult)
    w2 = small.tile([P, 2, 17], F32, tag="w2")
    nc.vector.tensor_scalar(w2[:], ohc[:], -1e9, 1e9, OP.mult, OP.add)
    nc.vector.tensor_tensor(wc_[:], wc_[:], w2[:], OP.add)
    CPL = out_sb[:, 172:184].rearrange("p (s f) -> p s f", f=6)  # [P,2,6]
    lagi = small.tile([P, 2], F32, tag="lagi")
    nc.vector.tensor_reduce(lagi[:], wc_[:], AX.X, OP.min)
    nc.vector.tensor_scalar(CPL[:, :, 4], lagi[:], float(LAGS), None, OP.subtract)
    # mv = cmax / (sqrt(256 var_f)*sqrt(256 var_s) + eps)
    nf = small.tile([P, 2], F32, tag="nf")
    nc.scalar.activation(nf[:], zbn2[:, 0:2, 1], AF.Sqrt, scale=float(T))
    ns_ = small.tile([P, 2], F32, tag="ns")
    nc.scalar.activation(ns_[:], zbn2[:, 2:4, 1], AF.Sqrt, scale=float(T))
    den = small.tile([P, 2], F32, tag="den")
    nc.vector.tensor_tensor(den[:], nf[:], ns_[:], OP.mult)
    nc.vector.tensor_scalar(den[:], den[:], EPS, None, OP.add)
    nc.vector.reciprocal(den[:], den[:])
    nc.vector.tensor_tensor(CPL[:, :, 3], cmax[:], den[:], OP.mult)
    # |sz|max / (|fz|max + eps)
    zmax = small.tile([P, 4], F32, tag="zmax")
    nc.vector.tensor_reduce(zmax[:, 0:2], zf, AX.X, OP.max, apply_absolute_value=True)
    nc.vector.tensor_reduce(zmax[:, 2:4], zs, AX.X, OP.max, apply_absolute_value=True)
    fzr = small.tile([P, 2], F32, tag="fzr")
    nc.vector.tensor_scalar(fzr[:], zmax[:, 0:2], EPS, None, OP.add)
    nc.vector.reciprocal(fzr[:], fzr[:])
    nc.vector.tensor_tensor(CPL[:, :, 0], zmax[:, 2:4], fzr[:], OP.mult)
    # ratio = rms_s / (rms_f + eps)  (rms cols of summary: sig 4+i vs 0+i)
    rms12v = osumR[:, :, 2]
    rr = small.tile([P, 2], F32, tag="rr")
    nc.vector.tensor_scalar(rr[:], rms12v[:, 0:2], EPS, None, OP.add)
    nc.vector.reciprocal(rr[:], rr[:])
    ratio = small.tile([P, 2], F32, tag="ratio")
    nc.vector.tensor_tensor(ratio[:], rms12v[:, 4:6], rr[:], OP.mult)
    nc.scalar.copy(CPL[:, :, 1], ratio[:])
    # H ratio: heel locsum sig 2+i over 0+i
    Hls = out_sb[:, 124:148].rearrange("p (s f) -> p s f", f=6)[:, :, 1]
    hr = small.tile([P, 2], F32, tag="hr")
    nc.vector.tensor_scalar(hr[:], Hls[:, 0:2], EPS, None, OP.add)
    nc.vector.reciprocal(hr[:], hr[:])
    nc.vector.tensor_tensor(CPL[:, :, 2], Hls[:, 2:4], hr[:], OP.mult)
    # 0.5*(SP_s[4]/(SP_f[4]+eps) + 1 - ratio)
    spr = small.tile([P, 2], F32, tag="spr")
    nc.vector.tensor_scalar(spr[:], SPv[:, 0:2, 4], EPS, None, OP.add)
    nc.vector.reciprocal(spr[:], spr[:])
    nc.vector.tensor_tensor(spr[:], SPv[:, 2:4, 4], spr[:], OP.mult)
    nc.vector.tensor_tensor(spr[:], spr[:], ratio[:], OP.subtract)
    nc.vector.tensor_scalar(CPL[:, :, 5], spr[:], 0.5, 0.5, OP.mult, OP.add)

    if _cut():
        return
    # ---- horiz ------------------------------------------------------------
    HZ = out_sb[:, 184:196].rearrange("p (s f) -> p s f", f=6)  # [P,2,6]
    sqv = sq_foot[:].rearrange("p (g s) t -> p g s t", s=6)
    hsq = work.tile([P, 2, T], F32, tag="hsq")
    nc.vector.tensor_tensor(hsq[:], sqv[:, :, 0, :], sqv[:, :, 1, :], OP.add)
    h = work.tile([P, 2, T], F32, tag="h")
    nc.scalar.activation(h[:], hsq[:], AF.Sqrt)
    hb2 = mean_var(h, 2, T, "hb")
    hmm = small.tile([P, 2], F32, tag="hmm")
    nc.vector.tensor_tensor(hmm[:], hb2[:, :, 0], hb2[:, :, 0], OP.mult)
    nc.vector.tensor_tensor(hmm[:], hmm[:], hb2[:, :, 1], OP.add)
    hrms = small.tile([P, 2], F32, tag="hrms")
    nc.scalar.activation(hrms[:], hmm[:], AF.Sqrt)
    nc.scalar.copy(HZ[:, :, 0], hrms[:])
    t16h = small.tile([P, 2, 16], F32, tag="t16h")
    for s in range(2):
        reph = work.tile([P, T], F32, tag="u0", bufs=2)
        nc.vector.max(t16h[:, s, 0:8], hsq[:, s, :])
        nc.vector.match_replace(reph[:], t16h[:, s, 0:8], hsq[:, s, :], -1.0)
        nc.vector.max(t16h[:, s, 8:16], reph[:])
    rootsh = small.tile([P, 2, 3], F32, tag="rootsh")
    nc.scalar.copy(rootsh[:, :, 0], t16h[:, :, 13])
    nc.scalar.copy(rootsh[:, :, 1], t16h[:, :, 12])
    nc.scalar.copy(rootsh[:, :, 2], t16h[:, :, 0])
    nc.scalar.activation(rootsh[:], rootsh[:], AF.Sqrt)  # note scale=1 here
    nc.scalar.copy(HZ[:, :, 1], rootsh[:, :, 2])         # max
    dqh = small.tile([P, 2], F32, tag="dqh")
    nc.vector.tensor_tensor(dqh[:], rootsh[:, :, 1], rootsh[:, :, 0], OP.subtract)
    nc.vector.scalar_tensor_tensor(HZ[:, :, 2], dqh[:], 0.25, rootsh[:, :, 0],
                                   OP.mult, OP.add)      # q95
    jkh = work.tile([P, 2, T - 1], F32, tag="jkh")
    nc.vector.tensor_tensor(jkh[:], h[:, :, 1:], h[:, :, :-1], OP.subtract)
    nc.vector.tensor_reduce(HZ[:, :, 3], jkh[:], AX.X, OP.max, apply_absolute_value=True)
    jhb2 = mean_var(jkh, 2, T - 1, "jhb")
    jhm = small.tile([P, 2], F32, tag="jhm")
    nc.vector.tensor_tensor(jhm[:], jhb2[:, :, 0], jhb2[:, :, 0], OP.mult)
    nc.vector.tensor_tensor(jhm[:], jhm[:], jhb2[:, :, 1], OP.add)
    nc.scalar.activation(HZ[:, :, 4], jhm[:], AF.Sqrt)
    mz = small.tile([P, 2], F32, tag="mz")
    for s in range(2):
        nc.scalar.activation(work.tile([P, T], F32, tag="junk", name="junkf", bufs=4)[:],
                             zf[:, s, :], AF.Abs, accum_out=mz[:, s:s + 1])
    nc.vector.tensor_scalar(mz[:], mz[:], 1.0 / T, EPS, OP.mult, OP.add)
    nc.vector.reciprocal(mz[:], mz[:])
    nc.vector.tensor_tensor(HZ[:, :, 5], hrms[:], mz[:], OP.mult)

    if _cut():
        return
    # ---- asym -------------------------------------------------------------
    lnm = small.tile([P, NSIG], F32, tag="lnm")
    nc.scalar.activation(lnm[:], osumR[:, :, 3], AF.Ln, bias=eps_sb[:])
    lnr = small.tile([P, NSIG], F32, tag="lnr")
    nc.scalar.activation(lnr[:], osumR[:, :, 2], AF.Ln, bias=eps_sb[:])
    lnh = small.tile([P, 4], F32, tag="lnh")
    nc.scalar.activation(lnh[:], Hls[:], AF.Ln, bias=eps_sb[:])
    AS = out_sb[:, 196:208]
    lm2 = lnm[:, 0:8].rearrange("p (a b) -> p a b", b=2)
    dm = small.tile([P, 4], F32, tag="dm")
    nc.vector.tensor_tensor(dm[:], lm2[:, :, 0], lm2[:, :, 1], OP.subtract)
    nc.scalar.activation(AS.rearrange("p (a b) -> p a b", b=2)[:, 0:4, 0], dm[:], AF.Abs)
    lr2 = lnr[:].rearrange("p (a b) -> p a b", b=2)
    dr = small.tile([P, 6], F32, tag="dr")
    nc.vector.tensor_tensor(dr[:], lr2[:, :, 0], lr2[:, :, 1], OP.subtract)
    absr = small.tile([P, 6], F32, tag="absr")
    nc.scalar.activation(absr[:], dr[:], AF.Abs)
    nc.scalar.copy(AS.rearrange("p (a b) -> p a b", b=2)[:, 0:4, 1], absr[:, 0:4])
    nc.scalar.copy(AS[:, 8:10], absr[:, 4:6])
    lh2 = lnh[:].rearrange("p (a b) -> p a b", b=2)
    dh = small.tile([P, 2], F32, tag="dh")
    nc.vector.tensor_tensor(dh[:], lh2[:, :, 0], lh2[:, :, 1], OP.subtract)
    nc.scalar.activation(AS[:, 10:12], dh[:], AF.Abs)

    # ---- store ------------------------------------------------------------
    nc.sync.dma_start(out_d[r0:r0 + P], out_sb[:])


def build_program(b_core):
    assert b_core % 128 == 0
    nc = bacc.Bacc("TRN2", target_bir_lowering=False, debug=False,
                   enable_asserts=False, num_devices=1)
    foot_d = nc.dram_tensor("foot", [b_core, 12, T], F32, kind="ExternalInput").ap()
    shank_d = nc.dram_tensor("shank", [b_core, 12, T], F32, kind="ExternalInput").ap()
    thigh_d = nc.dram_tensor("thigh", [b_core, 12, T], F32, kind="ExternalInput").ap()
    out_d = nc.dram_tensor("out", [b_core, 208], F32, kind="ExternalOutput").ap()

    Wr, ident, iota_iqr, iota_ph = _consts()
    W_dram = nc.inline_tensor(Wr, "w_dft")
    id_dram = nc.inline_tensor(ident, "ident")
    iota_iqr_dram = nc.inline_tensor(iota_iqr, "iota_iqr")
    iota_ph_dram = nc.inline_tensor(iota_ph, "iota_ph")

    with tile.TileContext(nc) as tc:
        from contextlib import ExitStack
        with ExitStack() as ctx:
            cpool = ctx.enter_context(tc.tile_pool(name="consts", bufs=1))
            iosb = ctx.enter_context(tc.tile_pool(name="io", bufs=2))
            psum = ctx.enter_context(tc.tile_pool(name="psum", bufs=2, space="PSUM"))
            work = ctx.enter_context(tc.tile_pool(name="work", bufs=1))
            small = ctx.enter_context(tc.tile_pool(name="small", bufs=1))
            W_sb = cpool.tile([128, 2, 2 * NBIN], F32, tag="wdft")
            nc.sync.dma_start(W_sb[:], W_dram.ap())
            id_sb = cpool.tile([128, 128], F32, tag="ident")
            nc.sync.dma_start(id_sb[:], id_dram.ap())
            iota_iqr_sb = cpool.tile([128, 12, 32], F32, tag="iotaq")
            nc.sync.dma_start(iota_iqr_sb[:], iota_iqr_dram.ap())
            iota_ph_sb = cpool.tile([128, 4, 115], F32, tag="iotap")
            nc.sync.dma_start(iota_ph_sb[:], iota_ph_dram.ap())
            eps_sb = cpool.tile([128, 1], F32, tag="epsc")
            nc.vector.memset(eps_sb[:], EPS)
            zeros_sb = cpool.tile([128, 160], F32, tag="zeros")
            nc.vector.memset(zeros_sb[:], 0.0)
            pools = (iosb, psum, work, small)
            consts = (W_sb, id_sb, eps_sb, zeros_sb, iota_iqr_sb, iota_ph_sb)
            for ti in range(b_core // 128):
                build_tile(tc, pools, consts,
                           (foot_d, shank_d, thigh_d), out_d, ti)
    nc.compile()
    return nc


_CACHE = {}


def _get_program(b_core):
    if b_core not in _CACHE:
        _CACHE[b_core] = build_program(b_core)
    return _CACHE[b_core]


def kernel(foot, shank, thigh):
    B = foot.shape[0]
    NCORES = 8
    bc = B // NCORES
    nc = _get_program(bc)
    in_maps = [{
        "foot": np.ascontiguousarray(foot[i * bc:(i + 1) * bc]),
        "shank": np.ascontiguousarray(shank[i * bc:(i + 1) * bc]),
        "thigh": np.ascontiguousarray(thigh[i * bc:(i + 1) * bc]),
    } for i in range(NCORES)]
    res = run_bass_kernel_spmd(nc, in_maps, list(range(NCORES)))
    return np.concatenate([res.results[i]["out"] for i in range(NCORES)], 0)

